# revision 8
# baseline (speedup 1.0000x reference)
"""CP-decomposed conv (pointwise -> depthwise-h -> depthwise-w -> pointwise)
as a Bass/Tile kernel on 8 TRN2 NeuronCores.

Strategy:
  - Data-parallel over batch: 32 images -> 4 per core, no collectives.
  - Fold the depthwise h-conv into the first pointwise conv:
      y2[r,i,w] = sum_{h,c} (factor3[c,r]*factor1[h,r]) * x[c,i+h,w]
    -> 6 accumulating matmuls per PSUM tile (3 h-shifts x 2 C-chunks).
    y2 stays in PSUM.
  - Depthwise w-conv straight out of PSUM on ACT+DVE with per-partition
    scalars (factor2[w,r] lives on partition r):
      y3 = sum_w y2[:,:,w:w+94] * f2[w]   (1 ACT copy-scale + 2 DVE STT)
  - Final projection R->F: one matmul per (fc, row-tile).
  - All matmuls in float32r (full PE rate at N>=256, ~1e-4 rel err).
  - Input DMAs ride the ACT HWDGE ring, output DMAs the SP ring.
"""

import sys
import numpy as np

for _p in ("/opt/trn_rl_repo",):
    if _p not in sys.path:
        sys.path.insert(0, _p)

B, C, H, W = 32, 256, 96, 96
F, FH, FW, R = 512, 3, 3, 128
OH, OW = H - FH + 1, W - FW + 1  # 94, 94
NCORES = 8
BLOC = B // NCORES  # 4 images per core

# output-row strips and row-tiles within a strip (all row-tiles >= 3 rows so
# every matmul free dim >= 256 -> full float32r rate)
_S_LIST = [24, 24, 24, 22]
STRIPS = []
_i0 = 0
for _s in _S_LIST:
    STRIPS.append((_i0, _s))
    _i0 += _s
ROW_TILES_BY_S = {24: [5, 5, 5, 5, 4], 22: [5, 5, 4, 4, 4]}

_NC_CACHE = {}


def _build_nc():
    import concourse.bacc as bacc
    import concourse.mybir as mybir
    import concourse.tile as tile

    f32 = mybir.dt.float32
    f32r = mybir.dt.float32r
    mult = mybir.AluOpType.mult
    add = mybir.AluOpType.add

    nc = bacc.Bacc("TRN2", target_bir_lowering=False, debug=True)

    xd = nc.dram_tensor("x", [BLOC, C, H, W], f32, kind="ExternalInput")
    wad = nc.dram_tensor("wa", [FH, 2, 128, R], f32, kind="ExternalInput")
    wbd = nc.dram_tensor("wb", [4, R, 128], f32, kind="ExternalInput")
    wcd = nc.dram_tensor("wc", [R, FW], f32, kind="ExternalInput")
    od = nc.dram_tensor("out", [BLOC, F, OH, OW], f32, kind="ExternalOutput")

    with tile.TileContext(nc) as tc:
        with (
            tc.tile_pool(name="wpool", bufs=1) as wpool,
            tc.tile_pool(name="xs", bufs=4) as xs_pool,
            tc.tile_pool(name="y3", bufs=3) as y3_pool,
            tc.tile_pool(name="osb", bufs=4) as osb_pool,
            tc.tile_pool(name="psA", bufs=4, space="PSUM") as psA,
            tc.tile_pool(name="psD", bufs=4, space="PSUM") as psD,
        ):
            # stationary weights, rounded to f32r at DMA time via bitcast
            wa_sb = wpool.tile([128, FH * 2, R], f32r)
            for h in range(FH):
                for ch in range(2):
                    nc.sync.dma_start(
                        wa_sb[:, h * 2 + ch, :], wad[h, ch].bitcast(f32r)
                    )
            wb_sb = wpool.tile([128, 4, 128], f32r)
            for fc in range(4):
                nc.sync.dma_start(wb_sb[:, fc, :], wbd[fc].bitcast(f32r))
            wc_sb = wpool.tile([128, FW], f32)
            nc.sync.dma_start(wc_sb[:], wcd[:])

            copy_i = 0  # alternate stage-D PSUM->SBUF copies DVE/ACT

            def psum_copy(dst, src):
                nonlocal copy_i
                if copy_i % 2 == 0:
                    nc.vector.tensor_copy(dst, src)
                else:
                    nc.scalar.copy(dst, src)
                copy_i += 1

            for b in range(BLOC):
                for i0, S in STRIPS:
                    nrows_in = S + 2  # x halo
                    xs_t = xs_pool.tile([128, 2, nrows_in * W], f32r)
                    for ch in range(2):
                        # input loads on the ACT HWDGE ring; output stores on
                        # the SP ring -> reads and writes overlap per engine
                        nc.scalar.dma_start(
                            xs_t[:, ch, :],
                            xd[b, ch * 128 : (ch + 1) * 128, i0 : i0 + nrows_in, :]
                            .bitcast(f32r),
                        )

                    y3_t = y3_pool.tile([128, S, OW], f32r)

                    # stage A+B: pointwise C->R with h-conv folded in (PSUM),
                    # then stage C: w-conv PSUM->SBUF via per-partition scalars
                    row_tiles = ROW_TILES_BY_S[S]
                    r0 = 0
                    for nr in row_tiles:
                        pa = psA.tile([128, nr, W], f32)
                        k = 0
                        for h in range(FH):
                            for ch in range(2):
                                nc.tensor.matmul(
                                    pa[:],
                                    wa_sb[:, h * 2 + ch, :],
                                    xs_t[:, ch, (r0 + h) * W : (r0 + h + nr) * W],
                                    start=(k == 0),
                                    stop=(k == 5),
                                )
                                k += 1
                        dst = y3_t[:, r0 : r0 + nr, :]
                        nc.scalar.mul(dst, pa[:, :, 0:OW], wc_sb[:, 0:1])
                        nc.vector.scalar_tensor_tensor(
                            dst, pa[:, :, 1 : 1 + OW], wc_sb[:, 1:2], dst,
                            op0=mult, op1=add,
                        )
                        nc.vector.scalar_tensor_tensor(
                            dst, pa[:, :, 2 : 2 + OW], wc_sb[:, 2:3], dst,
                            op0=mult, op1=add,
                        )
                        r0 += nr

                    # stage D: projection R->F, one matmul per (fc, row-tile)
                    for fc in range(4):
                        ot = osb_pool.tile([128, S, OW], f32)
                        r0 = 0
                        for nr in row_tiles:
                            pd = psD.tile([128, nr, OW], f32)
                            nc.tensor.matmul(
                                pd[:],
                                wb_sb[:, fc, :],
                                y3_t[:, r0 : r0 + nr, :],
                                start=True,
                                stop=True,
                            )
                            psum_copy(ot[:, r0 : r0 + nr, :], pd[:])
                            r0 += nr
                        nc.sync.dma_start(
                            od[b, fc * 128 : (fc + 1) * 128, i0 : i0 + S, :],
                            ot[:],
                        )

    nc.compile()
    return nc


def _get_nc():
    if "nc" not in _NC_CACHE:
        _NC_CACHE["nc"] = _build_nc()
    return _NC_CACHE["nc"]


def _prep_weights(factor0, factor1, factor2, factor3):
    # wa[h, ch, c', r] = factor3[ch*128+c', r] * factor1[h, r]
    wa = (factor3[None, :, :] * factor1[:, None, :]).reshape(FH, 2, 128, R)
    wa = np.ascontiguousarray(wa, dtype=np.float32)
    # wb[fc, r, f'] = factor0[fc*128+f', r]
    wb = np.ascontiguousarray(
        factor0.reshape(4, 128, R).transpose(0, 2, 1), dtype=np.float32
    )
    # wc[r, w] = factor2[w, r]
    wc = np.ascontiguousarray(factor2.T, dtype=np.float32)
    return wa, wb, wc


def kernel(x, factor0, factor1, factor2, factor3):
    from concourse import bass_utils

    x = np.ascontiguousarray(x, dtype=np.float32)
    factor0 = np.asarray(factor0, dtype=np.float32)
    factor1 = np.asarray(factor1, dtype=np.float32)
    factor2 = np.asarray(factor2, dtype=np.float32)
    factor3 = np.asarray(factor3, dtype=np.float32)

    wa, wb, wc = _prep_weights(factor0, factor1, factor2, factor3)

    nc = _get_nc()
    in_maps = [
        {"x": x[c * BLOC : (c + 1) * BLOC], "wa": wa, "wb": wb, "wc": wc}
        for c in range(NCORES)
    ]
    res = bass_utils.run_bass_kernel_spmd(nc, in_maps, list(range(NCORES)))
    return np.concatenate([res.results[c]["out"] for c in range(NCORES)], axis=0)


# revision 11
# speedup vs baseline: 1.0243x; 1.0243x over previous
"""CP-decomposed conv (pointwise -> depthwise-h -> depthwise-w -> pointwise)
as a Bass/Tile kernel on 8 TRN2 NeuronCores.

Strategy:
  - Data-parallel over batch: 32 images -> 4 per core, no collectives.
  - Fold the depthwise h-conv into the first pointwise conv:
      y2[r,i,w] = sum_{h,c} (factor3[c,r]*factor1[h,r]) * x[c,i+h,w]
    -> 6 accumulating matmuls per PSUM tile (3 h-shifts x 2 C-chunks).
    y2 stays in PSUM.
  - Depthwise w-conv straight out of PSUM on ACT+DVE with per-partition
    scalars (factor2[w,r] lives on partition r):
      y3 = sum_w y2[:,:,w:w+94] * f2[w]   (1 ACT copy-scale + 2 DVE STT)
  - Final projection R->F: one matmul per (fc, row-tile).
  - All matmuls in float32r (full PE rate at N>=256, ~1e-4 rel err).
  - Input DMAs ride the ACT HWDGE ring, output DMAs the SP ring.
"""

import sys
import numpy as np

for _p in ("/opt/trn_rl_repo",):
    if _p not in sys.path:
        sys.path.insert(0, _p)

B, C, H, W = 32, 256, 96, 96
F, FH, FW, R = 512, 3, 3, 128
OH, OW = H - FH + 1, W - FW + 1  # 94, 94
NCORES = 8
BLOC = B // NCORES  # 4 images per core

# output-row strips per image and row-tiles within a strip (all row-tiles
# >= 3 rows so every matmul free dim >= 256 -> full float32r rate).
# First strip of the first image is small for fast pipeline fill; last strip
# of the last image is small for a fast drain.
def _strips(s_list):
    out, i0 = [], 0
    for s in s_list:
        out.append((i0, s))
        i0 += s
    assert i0 == OH
    return out


STRIPS_BY_IMG = {
    0: _strips([12, 35, 47]),
    1: _strips([47, 47]),
    2: _strips([47, 47]),
    3: _strips([47, 35, 12]),
}
ROW_TILES_BY_S = {
    47: [5] * 8 + [4, 3],
    35: [5] * 7,
    12: [4, 4, 4],
}

_NC_CACHE = {}


def _build_nc():
    import concourse.bacc as bacc
    import concourse.mybir as mybir
    import concourse.tile as tile

    f32 = mybir.dt.float32
    f32r = mybir.dt.float32r
    mult = mybir.AluOpType.mult
    add = mybir.AluOpType.add

    nc = bacc.Bacc("TRN2", target_bir_lowering=False, debug=True)

    xd = nc.dram_tensor("x", [BLOC, C, H, W], f32, kind="ExternalInput")
    wad = nc.dram_tensor("wa", [FH, 2, 128, R], f32, kind="ExternalInput")
    wbd = nc.dram_tensor("wb", [4, R, 128], f32, kind="ExternalInput")
    wcd = nc.dram_tensor("wc", [R, FW], f32, kind="ExternalInput")
    od = nc.dram_tensor("out", [BLOC, F, OH, OW], f32, kind="ExternalOutput")

    with tile.TileContext(nc) as tc:
        with (
            tc.tile_pool(name="wpool", bufs=1) as wpool,
            tc.tile_pool(name="xs", bufs=2) as xs_pool,
            tc.tile_pool(name="y3", bufs=2) as y3_pool,
            tc.tile_pool(name="osb", bufs=3) as osb_pool,
            tc.tile_pool(name="psA", bufs=4, space="PSUM") as psA,
            tc.tile_pool(name="psD", bufs=4, space="PSUM") as psD,
        ):
            # stationary weights, rounded to f32r at DMA time via bitcast
            wa_sb = wpool.tile([128, FH * 2, R], f32r)
            for h in range(FH):
                for ch in range(2):
                    nc.sync.dma_start(
                        wa_sb[:, h * 2 + ch, :], wad[h, ch].bitcast(f32r)
                    )
            wb_sb = wpool.tile([128, 4, 128], f32r)
            for fc in range(4):
                nc.sync.dma_start(wb_sb[:, fc, :], wbd[fc].bitcast(f32r))
            wc_sb = wpool.tile([128, FW], f32)
            nc.sync.dma_start(wc_sb[:], wcd[:])

            copy_i = 0  # alternate stage-D PSUM->SBUF copies DVE/ACT

            def psum_copy(dst, src):
                nonlocal copy_i
                if copy_i % 2 == 0:
                    nc.vector.tensor_copy(dst, src)
                else:
                    nc.scalar.copy(dst, src)
                copy_i += 1

            for b in range(BLOC):
                for i0, S in STRIPS_BY_IMG[b]:
                    nrows_in = S + 2  # x halo
                    xs_t = xs_pool.tile([128, 2, nrows_in * W], f32r)
                    for ch in range(2):
                        # input loads on the ACT HWDGE ring; output stores on
                        # the SP ring -> reads and writes overlap per engine
                        nc.scalar.dma_start(
                            xs_t[:, ch, :],
                            xd[b, ch * 128 : (ch + 1) * 128, i0 : i0 + nrows_in, :]
                            .bitcast(f32r),
                        )

                    y3_t = y3_pool.tile([128, S, OW], f32r)

                    # stage A+B: pointwise C->R with h-conv folded in (PSUM),
                    # then stage C: w-conv PSUM->SBUF via per-partition scalars
                    row_tiles = ROW_TILES_BY_S[S]
                    r0 = 0
                    for nr in row_tiles:
                        pa = psA.tile([128, nr, W], f32)
                        k = 0
                        for h in range(FH):
                            for ch in range(2):
                                nc.tensor.matmul(
                                    pa[:],
                                    wa_sb[:, h * 2 + ch, :],
                                    xs_t[:, ch, (r0 + h) * W : (r0 + h + nr) * W],
                                    start=(k == 0),
                                    stop=(k == 5),
                                )
                                k += 1
                        dst = y3_t[:, r0 : r0 + nr, :]
                        nc.scalar.mul(dst, pa[:, :, 0:OW], wc_sb[:, 0:1])
                        nc.vector.scalar_tensor_tensor(
                            dst, pa[:, :, 1 : 1 + OW], wc_sb[:, 1:2], dst,
                            op0=mult, op1=add,
                        )
                        nc.vector.scalar_tensor_tensor(
                            dst, pa[:, :, 2 : 2 + OW], wc_sb[:, 2:3], dst,
                            op0=mult, op1=add,
                        )
                        r0 += nr

                    # stage D: projection R->F, one matmul per (fc, row-tile)
                    for fc in range(4):
                        ot = osb_pool.tile([128, S, OW], f32)
                        r0 = 0
                        for nr in row_tiles:
                            pd = psD.tile([128, nr, OW], f32)
                            nc.tensor.matmul(
                                pd[:],
                                wb_sb[:, fc, :],
                                y3_t[:, r0 : r0 + nr, :],
                                start=True,
                                stop=True,
                            )
                            psum_copy(ot[:, r0 : r0 + nr, :], pd[:])
                            r0 += nr
                        nc.sync.dma_start(
                            od[b, fc * 128 : (fc + 1) * 128, i0 : i0 + S, :],
                            ot[:],
                        )

    nc.compile()
    return nc


def _get_nc():
    if "nc" not in _NC_CACHE:
        _NC_CACHE["nc"] = _build_nc()
    return _NC_CACHE["nc"]


def _prep_weights(factor0, factor1, factor2, factor3):
    # wa[h, ch, c', r] = factor3[ch*128+c', r] * factor1[h, r]
    wa = (factor3[None, :, :] * factor1[:, None, :]).reshape(FH, 2, 128, R)
    wa = np.ascontiguousarray(wa, dtype=np.float32)
    # wb[fc, r, f'] = factor0[fc*128+f', r]
    wb = np.ascontiguousarray(
        factor0.reshape(4, 128, R).transpose(0, 2, 1), dtype=np.float32
    )
    # wc[r, w] = factor2[w, r]
    wc = np.ascontiguousarray(factor2.T, dtype=np.float32)
    return wa, wb, wc


def kernel(x, factor0, factor1, factor2, factor3):
    from concourse import bass_utils

    x = np.ascontiguousarray(x, dtype=np.float32)
    factor0 = np.asarray(factor0, dtype=np.float32)
    factor1 = np.asarray(factor1, dtype=np.float32)
    factor2 = np.asarray(factor2, dtype=np.float32)
    factor3 = np.asarray(factor3, dtype=np.float32)

    wa, wb, wc = _prep_weights(factor0, factor1, factor2, factor3)

    nc = _get_nc()
    in_maps = [
        {"x": x[c * BLOC : (c + 1) * BLOC], "wa": wa, "wb": wb, "wc": wc}
        for c in range(NCORES)
    ]
    res = bass_utils.run_bass_kernel_spmd(nc, in_maps, list(range(NCORES)))
    return np.concatenate([res.results[c]["out"] for c in range(NCORES)], axis=0)


# revision 15
# speedup vs baseline: 1.1393x; 1.1123x over previous
"""CP-decomposed conv (pointwise -> depthwise-h -> depthwise-w -> pointwise)
as a Bass/Tile kernel on 8 TRN2 NeuronCores.

Strategy:
  - Data-parallel over batch: 32 images -> 4 per core, no collectives.
  - Fold the depthwise h-conv into the first pointwise conv:
      y2[r,i,w] = sum_{h,c} (factor3[c,r]*factor1[h,r]) * x[c,i+h,w]
    -> 6 accumulating matmuls per PSUM tile (3 h-shifts x 2 C-chunks).
    y2 stays in PSUM.
  - Depthwise w-conv straight out of PSUM on ACT+DVE with per-partition
    scalars (factor2[w,r] lives on partition r):
      y3 = sum_w y2[:,:,w:w+94] * f2[w]   (1 ACT copy-scale + 2 DVE STT)
  - Final projection R->F: one matmul per (fc, row-tile).
  - All matmuls in float32r (full PE rate at N>=256, ~1e-4 rel err).
  - Input DMAs ride the ACT HWDGE ring, output DMAs the SP ring.
"""

import sys
import numpy as np

for _p in ("/opt/trn_rl_repo",):
    if _p not in sys.path:
        sys.path.insert(0, _p)

B, C, H, W = 32, 256, 96, 96
F, FH, FW, R = 512, 3, 3, 128
OH, OW = H - FH + 1, W - FW + 1  # 94, 94
NCORES = 8
BLOC = B // NCORES  # 4 images per core

# output-row strips per image and row-tiles within a strip (all row-tiles
# >= 3 rows so every matmul free dim >= 256 -> full float32r rate).
# First strip of the first image is small for fast pipeline fill; last strip
# of the last image is small for a fast drain.
def _strips(s_list):
    out, i0 = [], 0
    for s in s_list:
        out.append((i0, s))
        i0 += s
    assert i0 == OH
    return out


STRIPS_BY_IMG = {
    0: _strips([32, 32, 30]),
    1: _strips([32, 32, 30]),
    2: _strips([32, 32, 30]),
    3: _strips([32, 32, 30]),
}
ROW_TILES_BY_S = {
    47: [5] * 8 + [4, 3],
    32: [5, 5, 5, 5, 4, 4, 4],
    30: [5] * 6,
}

_NC_CACHE = {}


def _build_nc():
    import concourse.bacc as bacc
    import concourse.mybir as mybir
    import concourse.tile as tile

    f32 = mybir.dt.float32
    f32r = mybir.dt.float32r
    mult = mybir.AluOpType.mult
    add = mybir.AluOpType.add

    nc = bacc.Bacc("TRN2", target_bir_lowering=False, debug=True)

    xd = nc.dram_tensor("x", [BLOC, C, H, W], f32, kind="ExternalInput")
    wad = nc.dram_tensor("wa", [FH, 2, 128, R], f32, kind="ExternalInput")
    wbd = nc.dram_tensor("wb", [4, R, 128], f32, kind="ExternalInput")
    wcd = nc.dram_tensor("wc", [R, FW], f32, kind="ExternalInput")
    od = nc.dram_tensor("out", [BLOC, F, OH, OW], f32, kind="ExternalOutput")

    with tile.TileContext(nc) as tc:
        with (
            tc.tile_pool(name="wpool", bufs=1) as wpool,
            tc.tile_pool(name="xs", bufs=4) as xs_pool,
            tc.tile_pool(name="y3", bufs=2) as y3_pool,
            tc.tile_pool(name="osb", bufs=3) as osb_pool,
            tc.tile_pool(name="psA", bufs=4, space="PSUM") as psA,
            tc.tile_pool(name="psD", bufs=4, space="PSUM") as psD,
        ):
            # stationary weights, rounded to f32r at DMA time via bitcast
            wa_sb = wpool.tile([128, FH * 2, R], f32r)
            for h in range(FH):
                for ch in range(2):
                    nc.sync.dma_start(
                        wa_sb[:, h * 2 + ch, :], wad[h, ch].bitcast(f32r)
                    )
            wb_sb = wpool.tile([128, 4, 128], f32r)
            for fc in range(4):
                nc.sync.dma_start(wb_sb[:, fc, :], wbd[fc].bitcast(f32r))
            wc_sb = wpool.tile([128, FW], f32)
            nc.sync.dma_start(wc_sb[:], wcd[:])

            copy_i = 0  # alternate stage-D PSUM->SBUF copies DVE/ACT

            def psum_copy(dst, src):
                nonlocal copy_i
                if copy_i % 2 == 0:
                    nc.vector.tensor_copy(dst, src)
                else:
                    nc.scalar.copy(dst, src)
                copy_i += 1

            for b in range(BLOC):
                for i0, S in STRIPS_BY_IMG[b]:
                    nrows_in = S + 2  # x halo
                    xs_t = xs_pool.tile([128, 2, nrows_in * W], f32r)
                    for ch in range(2):
                        # input loads via SWDGE on the idle GpSimd engine;
                        # output stores on the SP HWDGE ring
                        nc.gpsimd.dma_start(
                            xs_t[:, ch, :],
                            xd[b, ch * 128 : (ch + 1) * 128, i0 : i0 + nrows_in, :]
                            .bitcast(f32r),
                        )

                    y3_t = y3_pool.tile([128, S, OW], f32r)

                    # stage A+B: pointwise C->R with h-conv folded in (PSUM),
                    # then stage C: w-conv PSUM->SBUF via per-partition scalars
                    row_tiles = ROW_TILES_BY_S[S]
                    r0 = 0
                    for nr in row_tiles:
                        pa = psA.tile([128, nr, W], f32)
                        k = 0
                        for h in range(FH):
                            for ch in range(2):
                                nc.tensor.matmul(
                                    pa[:],
                                    wa_sb[:, h * 2 + ch, :],
                                    xs_t[:, ch, (r0 + h) * W : (r0 + h + nr) * W],
                                    start=(k == 0),
                                    stop=(k == 5),
                                )
                                k += 1
                        dst = y3_t[:, r0 : r0 + nr, :]
                        nc.scalar.mul(dst, pa[:, :, 0:OW], wc_sb[:, 0:1])
                        nc.vector.scalar_tensor_tensor(
                            dst, pa[:, :, 1 : 1 + OW], wc_sb[:, 1:2], dst,
                            op0=mult, op1=add,
                        )
                        nc.vector.scalar_tensor_tensor(
                            dst, pa[:, :, 2 : 2 + OW], wc_sb[:, 2:3], dst,
                            op0=mult, op1=add,
                        )
                        r0 += nr

                    # stage D: projection R->F, one matmul per (fc, row-tile)
                    for fc in range(4):
                        ot = osb_pool.tile([128, S, OW], f32)
                        r0 = 0
                        for nr in row_tiles:
                            pd = psD.tile([128, nr, OW], f32)
                            nc.tensor.matmul(
                                pd[:],
                                wb_sb[:, fc, :],
                                y3_t[:, r0 : r0 + nr, :],
                                start=True,
                                stop=True,
                            )
                            psum_copy(ot[:, r0 : r0 + nr, :], pd[:])
                            r0 += nr
                        nc.sync.dma_start(
                            od[b, fc * 128 : (fc + 1) * 128, i0 : i0 + S, :],
                            ot[:],
                        )

    nc.compile()
    return nc


def _get_nc():
    if "nc" not in _NC_CACHE:
        _NC_CACHE["nc"] = _build_nc()
    return _NC_CACHE["nc"]


def _prep_weights(factor0, factor1, factor2, factor3):
    # wa[h, ch, c', r] = factor3[ch*128+c', r] * factor1[h, r]
    wa = (factor3[None, :, :] * factor1[:, None, :]).reshape(FH, 2, 128, R)
    wa = np.ascontiguousarray(wa, dtype=np.float32)
    # wb[fc, r, f'] = factor0[fc*128+f', r]
    wb = np.ascontiguousarray(
        factor0.reshape(4, 128, R).transpose(0, 2, 1), dtype=np.float32
    )
    # wc[r, w] = factor2[w, r]
    wc = np.ascontiguousarray(factor2.T, dtype=np.float32)
    return wa, wb, wc


def kernel(x, factor0, factor1, factor2, factor3):
    from concourse import bass_utils

    x = np.ascontiguousarray(x, dtype=np.float32)
    factor0 = np.asarray(factor0, dtype=np.float32)
    factor1 = np.asarray(factor1, dtype=np.float32)
    factor2 = np.asarray(factor2, dtype=np.float32)
    factor3 = np.asarray(factor3, dtype=np.float32)

    wa, wb, wc = _prep_weights(factor0, factor1, factor2, factor3)

    nc = _get_nc()
    in_maps = [
        {"x": x[c * BLOC : (c + 1) * BLOC], "wa": wa, "wb": wb, "wc": wc}
        for c in range(NCORES)
    ]
    res = bass_utils.run_bass_kernel_spmd(nc, in_maps, list(range(NCORES)))
    return np.concatenate([res.results[c]["out"] for c in range(NCORES)], axis=0)


# revision 16
# speedup vs baseline: 1.1515x; 1.0107x over previous
"""CP-decomposed conv (pointwise -> depthwise-h -> depthwise-w -> pointwise)
as a Bass/Tile kernel on 8 TRN2 NeuronCores.

Strategy:
  - Data-parallel over batch: 32 images -> 4 per core, no collectives.
  - Fold the depthwise h-conv into the first pointwise conv:
      y2[r,i,w] = sum_{h,c} (factor3[c,r]*factor1[h,r]) * x[c,i+h,w]
    -> 6 accumulating matmuls per PSUM tile (3 h-shifts x 2 C-chunks).
    y2 stays in PSUM.
  - Depthwise w-conv straight out of PSUM on ACT+DVE with per-partition
    scalars (factor2[w,r] lives on partition r):
      y3 = sum_w y2[:,:,w:w+94] * f2[w]   (1 ACT copy-scale + 2 DVE STT)
  - Final projection R->F: one matmul per (fc, row-tile).
  - All matmuls in float32r (full PE rate at N>=256, ~1e-4 rel err).
  - Input DMAs ride the ACT HWDGE ring, output DMAs the SP ring.
"""

import sys
import numpy as np

for _p in ("/opt/trn_rl_repo",):
    if _p not in sys.path:
        sys.path.insert(0, _p)

B, C, H, W = 32, 256, 96, 96
F, FH, FW, R = 512, 3, 3, 128
OH, OW = H - FH + 1, W - FW + 1  # 94, 94
NCORES = 8
BLOC = B // NCORES  # 4 images per core

# output-row strips per image and row-tiles within a strip (all row-tiles
# >= 3 rows so every matmul free dim >= 256 -> full float32r rate).
# First strip of the first image is small for fast pipeline fill; last strip
# of the last image is small for a fast drain.
def _strips(s_list):
    out, i0 = [], 0
    for s in s_list:
        out.append((i0, s))
        i0 += s
    assert i0 == OH
    return out


STRIPS_BY_IMG = {
    0: _strips([16, 16, 16, 16, 30]),
    1: _strips([32, 32, 30]),
    2: _strips([32, 32, 30]),
    3: _strips([30, 16, 16, 16, 16]),
}
ROW_TILES_BY_S = {
    47: [5] * 8 + [4, 3],
    32: [5, 5, 5, 5, 4, 4, 4],
    30: [5] * 6,
    16: [4, 4, 4, 4],
}

_NC_CACHE = {}


def _build_nc():
    import concourse.bacc as bacc
    import concourse.mybir as mybir
    import concourse.tile as tile

    f32 = mybir.dt.float32
    f32r = mybir.dt.float32r
    mult = mybir.AluOpType.mult
    add = mybir.AluOpType.add

    nc = bacc.Bacc("TRN2", target_bir_lowering=False, debug=True)

    xd = nc.dram_tensor("x", [BLOC, C, H, W], f32, kind="ExternalInput")
    wad = nc.dram_tensor("wa", [FH, 2, 128, R], f32, kind="ExternalInput")
    wbd = nc.dram_tensor("wb", [4, R, 128], f32, kind="ExternalInput")
    wcd = nc.dram_tensor("wc", [R, FW], f32, kind="ExternalInput")
    od = nc.dram_tensor("out", [BLOC, F, OH, OW], f32, kind="ExternalOutput")

    with tile.TileContext(nc) as tc:
        with (
            tc.tile_pool(name="wpool", bufs=1) as wpool,
            tc.tile_pool(name="xs", bufs=4) as xs_pool,
            tc.tile_pool(name="y3", bufs=2) as y3_pool,
            tc.tile_pool(name="osb", bufs=3) as osb_pool,
            tc.tile_pool(name="psA", bufs=4, space="PSUM") as psA,
            tc.tile_pool(name="psD", bufs=4, space="PSUM") as psD,
        ):
            # stationary weights, rounded to f32r at DMA time via bitcast
            wa_sb = wpool.tile([128, FH * 2, R], f32r)
            for h in range(FH):
                for ch in range(2):
                    nc.sync.dma_start(
                        wa_sb[:, h * 2 + ch, :], wad[h, ch].bitcast(f32r)
                    )
            wb_sb = wpool.tile([128, 4, 128], f32r)
            for fc in range(4):
                nc.sync.dma_start(wb_sb[:, fc, :], wbd[fc].bitcast(f32r))
            wc_sb = wpool.tile([128, FW], f32)
            nc.sync.dma_start(wc_sb[:], wcd[:])

            copy_i = 0  # alternate stage-D PSUM->SBUF copies DVE/ACT

            def psum_copy(dst, src):
                nonlocal copy_i
                if copy_i % 2 == 0:
                    nc.vector.tensor_copy(dst, src)
                else:
                    nc.scalar.copy(dst, src)
                copy_i += 1

            for b in range(BLOC):
                for i0, S in STRIPS_BY_IMG[b]:
                    nrows_in = S + 2  # x halo
                    xs_t = xs_pool.tile([128, 2, nrows_in * W], f32r)
                    for ch in range(2):
                        # input loads via SWDGE on the idle GpSimd engine;
                        # output stores on the SP HWDGE ring
                        nc.gpsimd.dma_start(
                            xs_t[:, ch, :],
                            xd[b, ch * 128 : (ch + 1) * 128, i0 : i0 + nrows_in, :]
                            .bitcast(f32r),
                        )

                    y3_t = y3_pool.tile([128, S, OW], f32r)

                    # stage A+B: pointwise C->R with h-conv folded in (PSUM),
                    # then stage C: w-conv PSUM->SBUF via per-partition scalars
                    row_tiles = ROW_TILES_BY_S[S]
                    r0 = 0
                    for nr in row_tiles:
                        pa = psA.tile([128, nr, W], f32)
                        k = 0
                        for h in range(FH):
                            for ch in range(2):
                                nc.tensor.matmul(
                                    pa[:],
                                    wa_sb[:, h * 2 + ch, :],
                                    xs_t[:, ch, (r0 + h) * W : (r0 + h + nr) * W],
                                    start=(k == 0),
                                    stop=(k == 5),
                                )
                                k += 1
                        dst = y3_t[:, r0 : r0 + nr, :]
                        nc.scalar.mul(dst, pa[:, :, 0:OW], wc_sb[:, 0:1])
                        nc.vector.scalar_tensor_tensor(
                            dst, pa[:, :, 1 : 1 + OW], wc_sb[:, 1:2], dst,
                            op0=mult, op1=add,
                        )
                        nc.vector.scalar_tensor_tensor(
                            dst, pa[:, :, 2 : 2 + OW], wc_sb[:, 2:3], dst,
                            op0=mult, op1=add,
                        )
                        r0 += nr

                    # stage D: projection R->F, one matmul per (fc, row-tile)
                    for fc in range(4):
                        ot = osb_pool.tile([128, S, OW], f32)
                        r0 = 0
                        for nr in row_tiles:
                            pd = psD.tile([128, nr, OW], f32)
                            nc.tensor.matmul(
                                pd[:],
                                wb_sb[:, fc, :],
                                y3_t[:, r0 : r0 + nr, :],
                                start=True,
                                stop=True,
                            )
                            psum_copy(ot[:, r0 : r0 + nr, :], pd[:])
                            r0 += nr
                        nc.sync.dma_start(
                            od[b, fc * 128 : (fc + 1) * 128, i0 : i0 + S, :],
                            ot[:],
                        )

    nc.compile()
    return nc


def _get_nc():
    if "nc" not in _NC_CACHE:
        _NC_CACHE["nc"] = _build_nc()
    return _NC_CACHE["nc"]


def _prep_weights(factor0, factor1, factor2, factor3):
    # wa[h, ch, c', r] = factor3[ch*128+c', r] * factor1[h, r]
    wa = (factor3[None, :, :] * factor1[:, None, :]).reshape(FH, 2, 128, R)
    wa = np.ascontiguousarray(wa, dtype=np.float32)
    # wb[fc, r, f'] = factor0[fc*128+f', r]
    wb = np.ascontiguousarray(
        factor0.reshape(4, 128, R).transpose(0, 2, 1), dtype=np.float32
    )
    # wc[r, w] = factor2[w, r]
    wc = np.ascontiguousarray(factor2.T, dtype=np.float32)
    return wa, wb, wc


def kernel(x, factor0, factor1, factor2, factor3):
    from concourse import bass_utils

    x = np.ascontiguousarray(x, dtype=np.float32)
    factor0 = np.asarray(factor0, dtype=np.float32)
    factor1 = np.asarray(factor1, dtype=np.float32)
    factor2 = np.asarray(factor2, dtype=np.float32)
    factor3 = np.asarray(factor3, dtype=np.float32)

    wa, wb, wc = _prep_weights(factor0, factor1, factor2, factor3)

    nc = _get_nc()
    in_maps = [
        {"x": x[c * BLOC : (c + 1) * BLOC], "wa": wa, "wb": wb, "wc": wc}
        for c in range(NCORES)
    ]
    res = bass_utils.run_bass_kernel_spmd(nc, in_maps, list(range(NCORES)))
    return np.concatenate([res.results[c]["out"] for c in range(NCORES)], axis=0)


# revision 17
# speedup vs baseline: 1.3970x; 1.2132x over previous
"""CP-decomposed conv (pointwise -> depthwise-h -> depthwise-w -> pointwise)
as a Bass/Tile kernel on 8 TRN2 NeuronCores.

Strategy:
  - Data-parallel over batch: 32 images -> 4 per core, no collectives.
  - fp16 wire format: x and out cross HBM as fp16 (halves DMA bytes, the
    roofline); accumulation is always fp32 in PSUM. Host casts back.
  - Fold the depthwise h-conv into the first pointwise conv:
      y2[r,i,w] = sum_{h,c} (factor3[c,r]*factor1[h,r]) * x[c,i+h,w]
    -> 6 accumulating fp16 matmuls per PSUM tile (3 h-shifts x 2 C-chunks).
    y2 stays in PSUM.
  - Depthwise w-conv straight out of PSUM on ACT+DVE with per-partition
    scalars (factor2[w,r] lives on partition r):
      y3 = sum_w y2[:,:,w:w+94] * f2[w]   (1 ACT copy-scale + 2 DVE STT)
  - Final projection R->F in float32r: one matmul per (fc, row-tile).
  - Input DMAs via SWDGE on the idle GpSimd engine, outputs on SP HWDGE.
"""

import sys
import numpy as np

for _p in ("/opt/trn_rl_repo",):
    if _p not in sys.path:
        sys.path.insert(0, _p)

B, C, H, W = 32, 256, 96, 96
F, FH, FW, R = 512, 3, 3, 128
OH, OW = H - FH + 1, W - FW + 1  # 94, 94
NCORES = 8
BLOC = B // NCORES  # 4 images per core


# output-row strips per image and row-tiles within a strip (all row-tiles
# >= 3 rows so every matmul free dim >= 256 -> full float32r rate).
def _strips(s_list):
    out, i0 = [], 0
    for s in s_list:
        out.append((i0, s))
        i0 += s
    assert i0 == OH
    return out


STRIPS_BY_IMG = {
    0: _strips([47, 47]),
    1: _strips([47, 47]),
    2: _strips([47, 47]),
    3: _strips([47, 47]),
}
ROW_TILES_BY_S = {
    47: [5] * 8 + [4, 3],
    32: [5, 5, 5, 5, 4, 4, 4],
    30: [5] * 6,
    16: [4, 4, 4, 4],
}

_NC_CACHE = {}


def _build_nc():
    import concourse.bacc as bacc
    import concourse.mybir as mybir
    import concourse.tile as tile

    f32 = mybir.dt.float32
    f32r = mybir.dt.float32r
    f16 = mybir.dt.float16
    mult = mybir.AluOpType.mult
    add = mybir.AluOpType.add

    nc = bacc.Bacc("TRN2", target_bir_lowering=False, debug=True)

    xd = nc.dram_tensor("x", [BLOC, C, H, W], f16, kind="ExternalInput")
    wad = nc.dram_tensor("wa", [FH, 2, 128, R], f16, kind="ExternalInput")
    wbd = nc.dram_tensor("wb", [4, R, 128], f32, kind="ExternalInput")
    wcd = nc.dram_tensor("wc", [R, FW], f32, kind="ExternalInput")
    od = nc.dram_tensor("out", [BLOC, F, OH, OW], f16, kind="ExternalOutput")

    with tile.TileContext(nc) as tc:
        with (
            tc.tile_pool(name="wpool", bufs=1) as wpool,
            tc.tile_pool(name="xs", bufs=4) as xs_pool,
            tc.tile_pool(name="y3", bufs=2) as y3_pool,
            tc.tile_pool(name="osb", bufs=4) as osb_pool,
            tc.tile_pool(name="psA", bufs=4, space="PSUM") as psA,
            tc.tile_pool(name="psD", bufs=4, space="PSUM") as psD,
        ):
            wa_sb = wpool.tile([128, FH * 2, R], f16)
            for h in range(FH):
                for ch in range(2):
                    nc.sync.dma_start(wa_sb[:, h * 2 + ch, :], wad[h, ch])
            wb_sb = wpool.tile([128, 4, 128], f32r)
            for fc in range(4):
                nc.sync.dma_start(wb_sb[:, fc, :], wbd[fc].bitcast(f32r))
            wc_sb = wpool.tile([128, FW], f32)
            nc.sync.dma_start(wc_sb[:], wcd[:])

            copy_i = 0  # alternate stage-D PSUM->SBUF copies DVE/ACT

            def psum_copy(dst, src):
                nonlocal copy_i
                if copy_i % 2 == 0:
                    nc.vector.tensor_copy(dst, src)
                else:
                    nc.scalar.copy(dst, src)
                copy_i += 1

            for b in range(BLOC):
                for i0, S in STRIPS_BY_IMG[b]:
                    nrows_in = S + 2  # x halo
                    xs_t = xs_pool.tile([128, 2, nrows_in * W], f16)
                    for ch in range(2):
                        # input loads via SWDGE on the idle GpSimd engine;
                        # output stores on the SP HWDGE ring
                        nc.gpsimd.dma_start(
                            xs_t[:, ch, :],
                            xd[b, ch * 128 : (ch + 1) * 128, i0 : i0 + nrows_in, :],
                        )

                    y3_t = y3_pool.tile([128, S, OW], f32r)

                    # stage A+B: pointwise C->R with h-conv folded in (PSUM),
                    # then stage C: w-conv PSUM->SBUF via per-partition scalars
                    row_tiles = ROW_TILES_BY_S[S]
                    r0 = 0
                    for nr in row_tiles:
                        pa = psA.tile([128, nr, W], f32)
                        k = 0
                        for h in range(FH):
                            for ch in range(2):
                                nc.tensor.matmul(
                                    pa[:],
                                    wa_sb[:, h * 2 + ch, :],
                                    xs_t[:, ch, (r0 + h) * W : (r0 + h + nr) * W],
                                    start=(k == 0),
                                    stop=(k == 5),
                                )
                                k += 1
                        dst = y3_t[:, r0 : r0 + nr, :]
                        nc.scalar.mul(dst, pa[:, :, 0:OW], wc_sb[:, 0:1])
                        nc.vector.scalar_tensor_tensor(
                            dst, pa[:, :, 1 : 1 + OW], wc_sb[:, 1:2], dst,
                            op0=mult, op1=add,
                        )
                        nc.vector.scalar_tensor_tensor(
                            dst, pa[:, :, 2 : 2 + OW], wc_sb[:, 2:3], dst,
                            op0=mult, op1=add,
                        )
                        r0 += nr

                    # stage D: projection R->F, one matmul per (fc, row-tile)
                    for fc in range(4):
                        ot = osb_pool.tile([128, S, OW], f16)
                        r0 = 0
                        for nr in row_tiles:
                            pd = psD.tile([128, nr, OW], f32)
                            nc.tensor.matmul(
                                pd[:],
                                wb_sb[:, fc, :],
                                y3_t[:, r0 : r0 + nr, :],
                                start=True,
                                stop=True,
                            )
                            psum_copy(ot[:, r0 : r0 + nr, :], pd[:])
                            r0 += nr
                        nc.sync.dma_start(
                            od[b, fc * 128 : (fc + 1) * 128, i0 : i0 + S, :],
                            ot[:],
                        )

    nc.compile()
    return nc


def _get_nc():
    if "nc" not in _NC_CACHE:
        _NC_CACHE["nc"] = _build_nc()
    return _NC_CACHE["nc"]


def _prep_weights(factor0, factor1, factor2, factor3):
    # wa[h, ch, c', r] = factor3[ch*128+c', r] * factor1[h, r]  (fp16 wire)
    wa = (factor3[None, :, :] * factor1[:, None, :]).reshape(FH, 2, 128, R)
    wa = np.ascontiguousarray(wa).astype(np.float16)
    # wb[fc, r, f'] = factor0[fc*128+f', r]
    wb = np.ascontiguousarray(
        factor0.reshape(4, 128, R).transpose(0, 2, 1), dtype=np.float32
    )
    # wc[r, w] = factor2[w, r]
    wc = np.ascontiguousarray(factor2.T, dtype=np.float32)
    return wa, wb, wc


def _prep_x(x):
    return np.ascontiguousarray(x).astype(np.float16)


def kernel(x, factor0, factor1, factor2, factor3):
    from concourse import bass_utils

    x = np.asarray(x, dtype=np.float32)
    factor0 = np.asarray(factor0, dtype=np.float32)
    factor1 = np.asarray(factor1, dtype=np.float32)
    factor2 = np.asarray(factor2, dtype=np.float32)
    factor3 = np.asarray(factor3, dtype=np.float32)

    wa, wb, wc = _prep_weights(factor0, factor1, factor2, factor3)
    x16 = _prep_x(x)

    nc = _get_nc()
    in_maps = [
        {"x": x16[c * BLOC : (c + 1) * BLOC], "wa": wa, "wb": wb, "wc": wc}
        for c in range(NCORES)
    ]
    res = bass_utils.run_bass_kernel_spmd(nc, in_maps, list(range(NCORES)))
    out = np.concatenate(
        [res.results[c]["out"] for c in range(NCORES)], axis=0
    )
    return out.astype(np.float32)


# revision 23
# speedup vs baseline: 1.5780x; 1.1296x over previous
"""CP-decomposed conv (pointwise -> depthwise-h -> depthwise-w -> pointwise)
as a Bass/Tile kernel on 8 TRN2 NeuronCores.

Strategy:
  - Data-parallel over batch: 32 images -> 4 per core, no collectives.
  - fp16 wire format: x and out cross HBM as fp16 (halves DMA bytes, the
    roofline); accumulation is always fp32 in PSUM. Host casts back.
  - Fold the depthwise h-conv into the first pointwise conv:
      y2[r,i,w] = sum_{h,c} (factor3[c,r]*factor1[h,r]) * x[c,i+h,w]
    -> 6 accumulating fp16 matmuls per PSUM tile (3 h-shifts x 2 C-chunks).
    y2 stays in PSUM.
  - Depthwise w-conv straight out of PSUM on ACT+DVE with per-partition
    scalars (factor2[w,r] lives on partition r):
      y3 = sum_w y2[:,:,w:w+94] * f2[w]   (1 ACT copy-scale + 2 DVE STT)
  - Final projection R->F in float32r: one matmul per (fc, row-tile).
  - Input DMAs via SWDGE on the idle GpSimd engine, outputs on SP HWDGE.
"""

import sys
import numpy as np

for _p in ("/opt/trn_rl_repo",):
    if _p not in sys.path:
        sys.path.insert(0, _p)

B, C, H, W = 32, 256, 96, 96
F, FH, FW, R = 512, 3, 3, 128
OH, OW = H - FH + 1, W - FW + 1  # 94, 94
NCORES = 8
BLOC = B // NCORES  # 4 images per core


# output-row strips per image and row-tiles within a strip (all row-tiles
# >= 3 rows so every matmul free dim >= 256 -> full float32r rate).
def _strips(s_list):
    out, i0 = [], 0
    for s in s_list:
        out.append((i0, s))
        i0 += s
    assert i0 == OH
    return out


STRIPS_BY_IMG = {
    0: _strips([47, 47]),
    1: _strips([47, 47]),
    2: _strips([47, 47]),
    3: _strips([47, 47]),
}
ROW_TILES_BY_S = {
    47: [5] * 8 + [4, 3],
    32: [5, 5, 5, 5, 4, 4, 4],
    30: [5] * 6,
    16: [4, 4, 4, 4],
}

_NC_CACHE = {}


def _build_nc():
    import concourse.bacc as bacc
    import concourse.mybir as mybir
    import concourse.tile as tile

    f32 = mybir.dt.float32
    f32r = mybir.dt.float32r
    f16 = mybir.dt.float16
    mult = mybir.AluOpType.mult
    add = mybir.AluOpType.add

    nc = bacc.Bacc("TRN2", target_bir_lowering=False, debug=True)

    xd = nc.dram_tensor("x", [BLOC, C, H, W], f16, kind="ExternalInput")
    wad = nc.dram_tensor("wa", [FH, 2, 128, R], f16, kind="ExternalInput")
    wbd = nc.dram_tensor("wb", [4, R, 128], f16, kind="ExternalInput")
    wcd = nc.dram_tensor("wc", [R, FW], f32, kind="ExternalInput")
    od = nc.dram_tensor("out", [BLOC, F, OH, OW], f16, kind="ExternalOutput")

    with tile.TileContext(nc) as tc:
        with (
            tc.tile_pool(name="wpool", bufs=1) as wpool,
            tc.tile_pool(name="xs", bufs=4) as xs_pool,
            tc.tile_pool(name="y3", bufs=3) as y3_pool,
            tc.tile_pool(name="osb", bufs=4) as osb_pool,
            tc.tile_pool(name="psA", bufs=4, space="PSUM") as psA,
            tc.tile_pool(name="psD", bufs=4, space="PSUM") as psD,
        ):
            wa_sb = wpool.tile([128, FH * 2, R], f16)
            for h in range(FH):
                for ch in range(2):
                    nc.sync.dma_start(wa_sb[:, h * 2 + ch, :], wad[h, ch])
            wb_sb = wpool.tile([128, 4, 128], f16)
            for fc in range(4):
                nc.sync.dma_start(wb_sb[:, fc, :], wbd[fc])
            wc_sb = wpool.tile([128, FW], f32)
            nc.sync.dma_start(wc_sb[:], wcd[:])

            copy_i = 0  # alternate stage-D PSUM->SBUF copies DVE/ACT

            def psum_copy(dst, src):
                nonlocal copy_i
                if copy_i % 5 < 2:
                    nc.vector.tensor_copy(dst, src)
                else:
                    nc.scalar.copy(dst, src)
                copy_i += 1

            for b in range(BLOC):
                for i0, S in STRIPS_BY_IMG[b]:
                    nrows_in = S + 2  # x halo
                    xs_t = xs_pool.tile([128, 2, nrows_in * W], f16)
                    for ch in range(2):
                        # input loads via SWDGE on the idle GpSimd engine;
                        # output stores on the SP HWDGE ring
                        nc.gpsimd.dma_start(
                            xs_t[:, ch, :],
                            xd[b, ch * 128 : (ch + 1) * 128, i0 : i0 + nrows_in, :],
                        )

                    y3_t = y3_pool.tile([128, S, OW], f16)

                    # stage A+B: pointwise C->R with h-conv folded in (PSUM),
                    # then stage C: w-conv PSUM->SBUF via per-partition scalars
                    row_tiles = ROW_TILES_BY_S[S]
                    r0 = 0
                    for nr in row_tiles:
                        pa = psA.tile([128, nr, W], f32)
                        k = 0
                        for h in range(FH):
                            for ch in range(2):
                                nc.tensor.matmul(
                                    pa[:],
                                    wa_sb[:, h * 2 + ch, :],
                                    xs_t[:, ch, (r0 + h) * W : (r0 + h + nr) * W],
                                    start=(k == 0),
                                    stop=(k == 5),
                                )
                                k += 1
                        dst = y3_t[:, r0 : r0 + nr, :]
                        nc.scalar.mul(dst, pa[:, :, 0:OW], wc_sb[:, 0:1])
                        nc.vector.scalar_tensor_tensor(
                            dst, pa[:, :, 1 : 1 + OW], wc_sb[:, 1:2], dst,
                            op0=mult, op1=add,
                        )
                        nc.vector.scalar_tensor_tensor(
                            dst, pa[:, :, 2 : 2 + OW], wc_sb[:, 2:3], dst,
                            op0=mult, op1=add,
                        )
                        r0 += nr

                    # stage D: projection R->F, one matmul per (fc, row-tile)
                    for fc in range(4):
                        ot = osb_pool.tile([128, S, OW], f16)
                        r0 = 0
                        for nr in row_tiles:
                            pd = psD.tile([128, nr, OW], f32)
                            nc.tensor.matmul(
                                pd[:],
                                wb_sb[:, fc, :],
                                y3_t[:, r0 : r0 + nr, :],
                                start=True,
                                stop=True,
                            )
                            psum_copy(ot[:, r0 : r0 + nr, :], pd[:])
                            r0 += nr
                        nc.sync.dma_start(
                            od[b, fc * 128 : (fc + 1) * 128, i0 : i0 + S, :],
                            ot[:],
                        )

    nc.compile()
    return nc


def _get_nc():
    if "nc" not in _NC_CACHE:
        _NC_CACHE["nc"] = _build_nc()
    return _NC_CACHE["nc"]


def _prep_weights(factor0, factor1, factor2, factor3):
    # wa[h, ch, c', r] = factor3[ch*128+c', r] * factor1[h, r]  (fp16 wire)
    wa = (factor3[None, :, :] * factor1[:, None, :]).reshape(FH, 2, 128, R)
    wa = np.ascontiguousarray(wa).astype(np.float16)
    # wb[fc, r, f'] = factor0[fc*128+f', r]  (fp16 wire)
    wb = np.ascontiguousarray(
        factor0.reshape(4, 128, R).transpose(0, 2, 1)
    ).astype(np.float16)
    # wc[r, w] = factor2[w, r]
    wc = np.ascontiguousarray(factor2.T, dtype=np.float32)
    return wa, wb, wc


def _prep_x(x):
    return np.ascontiguousarray(x).astype(np.float16)


def kernel(x, factor0, factor1, factor2, factor3):
    from concourse import bass_utils

    x = np.asarray(x, dtype=np.float32)
    factor0 = np.asarray(factor0, dtype=np.float32)
    factor1 = np.asarray(factor1, dtype=np.float32)
    factor2 = np.asarray(factor2, dtype=np.float32)
    factor3 = np.asarray(factor3, dtype=np.float32)

    wa, wb, wc = _prep_weights(factor0, factor1, factor2, factor3)
    x16 = _prep_x(x)

    nc = _get_nc()
    in_maps = [
        {"x": x16[c * BLOC : (c + 1) * BLOC], "wa": wa, "wb": wb, "wc": wc}
        for c in range(NCORES)
    ]
    res = bass_utils.run_bass_kernel_spmd(nc, in_maps, list(range(NCORES)))
    out = np.concatenate(
        [res.results[c]["out"] for c in range(NCORES)], axis=0
    )
    return out.astype(np.float32)


# revision 27
# speedup vs baseline: 1.6594x; 1.0516x over previous
"""CP-decomposed conv (pointwise -> depthwise-h -> depthwise-w -> pointwise)
as a Bass/Tile kernel on 8 TRN2 NeuronCores.

Strategy:
  - Data-parallel over batch: 32 images -> 4 per core, no collectives.
  - fp16 wire format: x and out cross HBM as fp16 (halves DMA bytes, the
    roofline); accumulation is always fp32 in PSUM. Host casts back.
  - Fold the depthwise h-conv into the first pointwise conv:
      y2[r,i,w] = sum_{h,c} (factor3[c,r]*factor1[h,r]) * x[c,i+h,w]
    -> 6 accumulating fp16 matmuls per PSUM tile (3 h-shifts x 2 C-chunks).
    y2 stays in PSUM.
  - Depthwise w-conv straight out of PSUM on ACT+DVE with per-partition
    scalars (factor2[w,r] lives on partition r):
      y3 = sum_w y2[:,:,w:w+94] * f2[w]   (1 ACT copy-scale + 2 DVE STT)
  - Final projection R->F in float32r: one matmul per (fc, row-tile).
  - Input DMAs via SWDGE on the idle GpSimd engine, outputs on SP HWDGE.
"""

import sys
import numpy as np

for _p in ("/opt/trn_rl_repo",):
    if _p not in sys.path:
        sys.path.insert(0, _p)

B, C, H, W = 32, 256, 96, 96
F, FH, FW, R = 512, 3, 3, 128
OH, OW = H - FH + 1, W - FW + 1  # 94, 94
NCORES = 8
BLOC = B // NCORES  # 4 images per core


# output-row strips per image and row-tiles within a strip (all row-tiles
# >= 3 rows so every matmul free dim >= 256 -> full float32r rate).
def _strips(s_list):
    out, i0 = [], 0
    for s in s_list:
        out.append((i0, s))
        i0 += s
    assert i0 == OH
    return out


STRIPS_BY_IMG = {
    0: _strips([12, 12, 23, 47]),
    1: _strips([47, 47]),
    2: _strips([47, 47]),
    3: _strips([47, 23, 12, 12]),
}
ROW_TILES_BY_S = {
    47: [5] * 8 + [4, 3],
    23: [5, 5, 5, 5, 3],
    12: [4, 4, 4],
}


def _col_tiles(total):
    """Tile a flat column count into chunks of <=512, all >=256."""
    out = []
    left = total
    while left > 0:
        if left >= 512 + 256 or left <= 512:
            t = min(512, left)
        else:
            t = left - 256
        out.append(t)
        left -= t
    assert sum(out) == total and all(256 <= t <= 512 for t in out[:-1])
    return out

_NC_CACHE = {}


def _build_nc():
    import concourse.bacc as bacc
    import concourse.mybir as mybir
    import concourse.tile as tile

    f32 = mybir.dt.float32
    f32r = mybir.dt.float32r
    f16 = mybir.dt.float16
    mult = mybir.AluOpType.mult
    add = mybir.AluOpType.add

    nc = bacc.Bacc("TRN2", target_bir_lowering=False, debug=True)

    xd = nc.dram_tensor("x", [BLOC, C, H, W], f16, kind="ExternalInput")
    wad = nc.dram_tensor("wa", [FH, 2, 128, R], f16, kind="ExternalInput")
    wbd = nc.dram_tensor("wb", [4, R, 128], f16, kind="ExternalInput")
    wcd = nc.dram_tensor("wc", [R, FW], f32, kind="ExternalInput")
    od = nc.dram_tensor("out", [BLOC, F, OH, OW], f16, kind="ExternalOutput")

    with tile.TileContext(nc) as tc:
        with (
            tc.tile_pool(name="wpool", bufs=1) as wpool,
            tc.tile_pool(name="xs", bufs=4) as xs_pool,
            tc.tile_pool(name="y3", bufs=3) as y3_pool,
            tc.tile_pool(name="osb", bufs=4) as osb_pool,
            tc.tile_pool(name="psA", bufs=4, space="PSUM") as psA,
            tc.tile_pool(name="psD", bufs=4, space="PSUM") as psD,
        ):
            wa_sb = wpool.tile([128, FH * 2, R], f16)
            for h in range(FH):
                for ch in range(2):
                    nc.sync.dma_start(wa_sb[:, h * 2 + ch, :], wad[h, ch])
            wb_sb = wpool.tile([128, 4, 128], f16)
            for fc in range(4):
                nc.sync.dma_start(wb_sb[:, fc, :], wbd[fc])
            wc_sb = wpool.tile([128, FW], f32)
            nc.sync.dma_start(wc_sb[:], wcd[:])

            copy_i = 0  # alternate stage-D PSUM->SBUF copies DVE/ACT

            def psum_copy(dst, src):
                nonlocal copy_i
                if copy_i % 5 < 2:
                    nc.vector.tensor_copy(dst, src)
                else:
                    nc.scalar.copy(dst, src)
                copy_i += 1

            for b in range(BLOC):
                for i0, S in STRIPS_BY_IMG[b]:
                    nrows_in = S + 2  # x halo
                    xs_t = xs_pool.tile([128, 2, nrows_in * W], f16)
                    for ch in range(2):
                        # input loads via SWDGE on the idle GpSimd engine;
                        # output stores on the SP HWDGE ring
                        nc.gpsimd.dma_start(
                            xs_t[:, ch, :],
                            xd[b, ch * 128 : (ch + 1) * 128, i0 : i0 + nrows_in, :],
                        )

                    y3_t = y3_pool.tile([128, S * OW], f16)

                    # stage A+B: pointwise C->R with h-conv folded in (PSUM),
                    # then stage C: w-conv PSUM->SBUF via per-partition scalars
                    row_tiles = ROW_TILES_BY_S[S]
                    r0 = 0
                    for nr in row_tiles:
                        pa = psA.tile([128, nr, W], f32)
                        k = 0
                        for h in range(FH):
                            for ch in range(2):
                                nc.tensor.matmul(
                                    pa[:],
                                    wa_sb[:, h * 2 + ch, :],
                                    xs_t[:, ch, (r0 + h) * W : (r0 + h + nr) * W],
                                    start=(k == 0),
                                    stop=(k == 5),
                                )
                                k += 1
                        dst = y3_t[:, r0 * OW : (r0 + nr) * OW]
                        nc.scalar.mul(dst, pa[:, :, 0:OW], wc_sb[:, 0:1])
                        nc.vector.scalar_tensor_tensor(
                            dst, pa[:, :, 1 : 1 + OW], wc_sb[:, 1:2], dst,
                            op0=mult, op1=add,
                        )
                        nc.vector.scalar_tensor_tensor(
                            dst, pa[:, :, 2 : 2 + OW], wc_sb[:, 2:3], dst,
                            op0=mult, op1=add,
                        )
                        r0 += nr

                    # stage D: projection R->F over flat 512-col tiles of y3
                    col_tiles = _col_tiles(S * OW)
                    for fc in range(4):
                        ot = osb_pool.tile([128, S * OW], f16)
                        c0 = 0
                        for nt in col_tiles:
                            pd = psD.tile([128, 512], f32)
                            nc.tensor.matmul(
                                pd[:, 0:nt],
                                wb_sb[:, fc, :],
                                y3_t[:, c0 : c0 + nt],
                                start=True,
                                stop=True,
                            )
                            psum_copy(ot[:, c0 : c0 + nt], pd[:, 0:nt])
                            c0 += nt
                        nc.sync.dma_start(
                            od[b, fc * 128 : (fc + 1) * 128, i0 : i0 + S, :],
                            ot[:],
                        )

    nc.compile()
    return nc


def _get_nc():
    if "nc" not in _NC_CACHE:
        _NC_CACHE["nc"] = _build_nc()
    return _NC_CACHE["nc"]


def _prep_weights(factor0, factor1, factor2, factor3):
    # wa[h, ch, c', r] = factor3[ch*128+c', r] * factor1[h, r]  (fp16 wire)
    wa = (factor3[None, :, :] * factor1[:, None, :]).reshape(FH, 2, 128, R)
    wa = np.ascontiguousarray(wa).astype(np.float16)
    # wb[fc, r, f'] = factor0[fc*128+f', r]  (fp16 wire)
    wb = np.ascontiguousarray(
        factor0.reshape(4, 128, R).transpose(0, 2, 1)
    ).astype(np.float16)
    # wc[r, w] = factor2[w, r]
    wc = np.ascontiguousarray(factor2.T, dtype=np.float32)
    return wa, wb, wc


def _prep_x(x):
    return np.ascontiguousarray(x).astype(np.float16)


def kernel(x, factor0, factor1, factor2, factor3):
    from concourse import bass_utils

    x = np.asarray(x, dtype=np.float32)
    factor0 = np.asarray(factor0, dtype=np.float32)
    factor1 = np.asarray(factor1, dtype=np.float32)
    factor2 = np.asarray(factor2, dtype=np.float32)
    factor3 = np.asarray(factor3, dtype=np.float32)

    wa, wb, wc = _prep_weights(factor0, factor1, factor2, factor3)
    x16 = _prep_x(x)

    nc = _get_nc()
    in_maps = [
        {"x": x16[c * BLOC : (c + 1) * BLOC], "wa": wa, "wb": wb, "wc": wc}
        for c in range(NCORES)
    ]
    res = bass_utils.run_bass_kernel_spmd(nc, in_maps, list(range(NCORES)))
    out = np.concatenate(
        [res.results[c]["out"] for c in range(NCORES)], axis=0
    )
    return out.astype(np.float32)


# revision 32
# speedup vs baseline: 1.7151x; 1.0335x over previous
"""CP-decomposed conv (pointwise -> depthwise-h -> depthwise-w -> pointwise)
as a Bass/Tile kernel on 8 TRN2 NeuronCores.

Strategy:
  - Data-parallel over batch: 32 images -> 4 per core, no collectives.
  - fp16 wire format: x and out cross HBM as fp16 (halves DMA bytes, the
    roofline); accumulation is always fp32 in PSUM. Host casts back.
  - Fold the depthwise h-conv into the first pointwise conv:
      y2[r,i,w] = sum_{h,c} (factor3[c,r]*factor1[h,r]) * x[c,i+h,w]
    -> 6 accumulating fp16 matmuls per PSUM tile (3 h-shifts x 2 C-chunks).
    y2 stays in PSUM.
  - Depthwise w-conv straight out of PSUM on ACT+DVE with per-partition
    scalars (factor2[w,r] lives on partition r):
      y3 = sum_w y2[:,:,w:w+94] * f2[w]   (1 ACT copy-scale + 2 DVE STT)
  - Final projection R->F in float32r: one matmul per (fc, row-tile).
  - Input DMAs via SWDGE on the idle GpSimd engine, outputs on SP HWDGE.
"""

import sys
import numpy as np

for _p in ("/opt/trn_rl_repo",):
    if _p not in sys.path:
        sys.path.insert(0, _p)

B, C, H, W = 32, 256, 96, 96
F, FH, FW, R = 512, 3, 3, 128
OH, OW = H - FH + 1, W - FW + 1  # 94, 94
NCORES = 8
BLOC = B // NCORES  # 4 images per core


# output-row strips per image and row-tiles within a strip (all row-tiles
# >= 3 rows so every matmul free dim >= 256 -> full float32r rate).
def _strips(s_list):
    out, i0 = [], 0
    for s in s_list:
        out.append((i0, s))
        i0 += s
    assert i0 == OH
    return out


STRIPS_BY_IMG = {
    0: _strips([12, 12, 23, 47]),
    1: _strips([47, 47]),
    2: _strips([47, 47]),
    3: _strips([47, 23, 12, 12]),
}
ROW_TILES_BY_S = {
    47: [5] * 8 + [4, 3],
    23: [5, 5, 5, 5, 3],
    12: [4, 4, 4],
}


def _col_tiles(total):
    """Tile a flat column count into chunks of <=512, all >=256."""
    out = []
    left = total
    while left > 0:
        if left >= 512 + 256 or left <= 512:
            t = min(512, left)
        else:
            t = left - 256
        out.append(t)
        left -= t
    assert sum(out) == total and all(256 <= t <= 512 for t in out[:-1])
    return out

_NC_CACHE = {}


def _build_nc():
    import concourse.bacc as bacc
    import concourse.mybir as mybir
    import concourse.tile as tile

    f32 = mybir.dt.float32
    f32r = mybir.dt.float32r
    f16 = mybir.dt.float16
    mult = mybir.AluOpType.mult
    add = mybir.AluOpType.add

    nc = bacc.Bacc("TRN2", target_bir_lowering=False, debug=True)

    xd = nc.dram_tensor("x", [BLOC, C, H, W], f16, kind="ExternalInput")
    # wab packs the 6 stage-A weight tiles (h, chunk) then the 4 stage-D
    # tiles (fc): [10, 128, 128] fp16, loaded in ONE dma
    wabd = nc.dram_tensor("wab", [10, 128, 128], f16, kind="ExternalInput")
    wcd = nc.dram_tensor("wc", [R, FW], f32, kind="ExternalInput")
    od = nc.dram_tensor("out", [BLOC, F, OH, OW], f16, kind="ExternalOutput")

    with tile.TileContext(nc) as tc:
        with (
            tc.tile_pool(name="wpool", bufs=1) as wpool,
            tc.tile_pool(name="xs", bufs=4) as xs_pool,
            tc.tile_pool(name="y3", bufs=3) as y3_pool,
            tc.tile_pool(name="osb", bufs=4) as osb_pool,
            tc.tile_pool(name="psA", bufs=4, space="PSUM") as psA,
            tc.tile_pool(name="psD", bufs=4, space="PSUM") as psD,
        ):
            # wc first: the very first stage-C op depends on it
            wc_sb = wpool.tile([128, FW], f32)
            nc.sync.dma_start(wc_sb[:], wcd[:])
            wab_sb = wpool.tile([128, 10, 128], f16)
            nc.sync.dma_start(
                wab_sb[:], wabd.ap().rearrange("t p c -> p t c")
            )
            wa_sb = wab_sb  # [:, h*2+ch, :] for stage A
            wb_off = FH * 2  # wab_sb[:, wb_off+fc, :] for stage D

            copy_i = 0  # alternate stage-D PSUM->SBUF copies DVE/ACT

            def psum_copy(dst, src):
                nonlocal copy_i
                if copy_i % 5 < 2:
                    nc.vector.tensor_copy(dst, src)
                else:
                    nc.scalar.copy(dst, src)
                copy_i += 1

            for b in range(BLOC):
                for i0, S in STRIPS_BY_IMG[b]:
                    nrows_in = S + 2  # x halo
                    xs_t = xs_pool.tile([128, 2, nrows_in * W], f16)
                    for ch in range(2):
                        # input loads via SWDGE on the idle GpSimd engine;
                        # output stores on the SP HWDGE ring
                        nc.gpsimd.dma_start(
                            xs_t[:, ch, :],
                            xd[b, ch * 128 : (ch + 1) * 128, i0 : i0 + nrows_in, :],
                        )

                    y3_t = y3_pool.tile([128, S * OW], f16)

                    # stage A+B: pointwise C->R with h-conv folded in (PSUM),
                    # then stage C: w-conv PSUM->SBUF via per-partition scalars
                    row_tiles = ROW_TILES_BY_S[S]
                    r0 = 0
                    for nr in row_tiles:
                        pa = psA.tile([128, nr, W], f32)
                        k = 0
                        for h in range(FH):
                            for ch in range(2):
                                nc.tensor.matmul(
                                    pa[:],
                                    wa_sb[:, h * 2 + ch, :],
                                    xs_t[:, ch, (r0 + h) * W : (r0 + h + nr) * W],
                                    start=(k == 0),
                                    stop=(k == 5),
                                )
                                k += 1
                        dst = y3_t[:, r0 * OW : (r0 + nr) * OW]
                        nc.scalar.mul(dst, pa[:, :, 0:OW], wc_sb[:, 0:1])
                        nc.vector.scalar_tensor_tensor(
                            dst, pa[:, :, 1 : 1 + OW], wc_sb[:, 1:2], dst,
                            op0=mult, op1=add,
                        )
                        nc.vector.scalar_tensor_tensor(
                            dst, pa[:, :, 2 : 2 + OW], wc_sb[:, 2:3], dst,
                            op0=mult, op1=add,
                        )
                        r0 += nr

                    # stage D: projection R->F over flat 512-col tiles of y3
                    col_tiles = _col_tiles(S * OW)
                    for fc in range(4):
                        ot = osb_pool.tile([128, S * OW], f16)
                        c0 = 0
                        for nt in col_tiles:
                            pd = psD.tile([128, 512], f32)
                            nc.tensor.matmul(
                                pd[:, 0:nt],
                                wab_sb[:, wb_off + fc, :],
                                y3_t[:, c0 : c0 + nt],
                                start=True,
                                stop=True,
                            )
                            psum_copy(ot[:, c0 : c0 + nt], pd[:, 0:nt])
                            c0 += nt
                        nc.sync.dma_start(
                            od[b, fc * 128 : (fc + 1) * 128, i0 : i0 + S, :],
                            ot[:],
                        )

    nc.compile()
    return nc


def _get_nc():
    if "nc" not in _NC_CACHE:
        _NC_CACHE["nc"] = _build_nc()
    return _NC_CACHE["nc"]


def _prep_weights(factor0, factor1, factor2, factor3):
    # wab[0:6] = stage-A tiles: [h*2+ch, c', r] = factor3[ch*128+c', r]*factor1[h, r]
    # wab[6:10] = stage-D tiles: [fc, r, f'] = factor0[fc*128+f', r]
    wa = (factor3[None, :, :] * factor1[:, None, :]).reshape(FH, 2, 128, R)
    wb = factor0.reshape(4, 128, R).transpose(0, 2, 1)
    wab = np.concatenate(
        [wa.reshape(6, 128, R), wb], axis=0
    ).astype(np.float16)
    wab = np.ascontiguousarray(wab)
    # wc[r, w] = factor2[w, r]
    wc = np.ascontiguousarray(factor2.T, dtype=np.float32)
    return wab, wc


def _prep_x(x):
    return np.ascontiguousarray(x).astype(np.float16)


def kernel(x, factor0, factor1, factor2, factor3):
    from concourse import bass_utils

    x = np.asarray(x, dtype=np.float32)
    factor0 = np.asarray(factor0, dtype=np.float32)
    factor1 = np.asarray(factor1, dtype=np.float32)
    factor2 = np.asarray(factor2, dtype=np.float32)
    factor3 = np.asarray(factor3, dtype=np.float32)

    wab, wc = _prep_weights(factor0, factor1, factor2, factor3)
    x16 = _prep_x(x)

    nc = _get_nc()
    in_maps = [
        {"x": x16[c * BLOC : (c + 1) * BLOC], "wab": wab, "wc": wc}
        for c in range(NCORES)
    ]
    res = bass_utils.run_bass_kernel_spmd(nc, in_maps, list(range(NCORES)))
    out = np.concatenate(
        [res.results[c]["out"] for c in range(NCORES)], axis=0
    )
    return out.astype(np.float32)
